# revision 1
# baseline (speedup 1.0000x reference)
"""CompressedLinear Trainium2 kernel.

Computes y = x @ (w_int8 * 0.01)^T + bias for
  x      [4, 32, 4096]  fp32
  w_int8 [11008, 4096]  int32 (int8 values)
  bias   [11008]        fp32
  y      [4, 32, 11008] fp32

Strategy (tensor-parallel over output rows, 8 NeuronCores):
- Host: transpose w to wT [4096, 11008] and shard the row dim into
  8 contiguous [4096, 1376] int32 shards so each core streams its shard
  with full-bandwidth, fully-contiguous DMAs. The int32 payload is
  preserved end-to-end: every core reads its full 22.5 MB from HBM; the
  int32->bf16 conversion happens inside the SDMA engines (SWDGE
  cast-DMA), which is exact for int8-valued data (|v| <= 128 is exactly
  representable in bf16) and costs no compute-engine cycles.
- Host: fold the 0.01 dequant scale into x, split x*0.01 into an exact
  bf16 (hi, lo) pair, transpose to [c, t] and swizzle into the SBUF
  layout the PE stationary operand consumes. Two bf16 matmul passes
  (hi + lo) recover fp32-level accuracy because bf16 x bf16 products
  accumulate exactly in fp32 PSUM (rel err ~2e-6 measured).
- Device, per core: stream the weight shard in a few large slabs
  (descending sizes, so the compute tail after the last DMA is tiny).
  For each 128-wide chunk of the contraction dim, load x^T[c] (hi, lo)
  as PE stationary and stream the bf16 weights; accumulate
  y[tokens, rows] in 3 PSUM banks (512/512/352 output rows). The bias
  enters PSUM via K=1 matmuls against a ones-vector (bias split into a
  bf16 hi/lo pair, exact to 2^-17). Per-bank PSUM drains alternate
  ScalarE/VectorE and each bank's output DMA starts as soon as its
  drain lands.
- Host: concatenate the 8 row-shards, reshape.
"""

from contextlib import ExitStack

import numpy as np
import ml_dtypes

ROWS, COLS = 11008, 4096
SCALE = 0.01
T = 128                      # tokens = 4*32
NCORES = 8
RPC = ROWS // NCORES         # 1376 rows per core
CCHUNK = 128                 # contraction tile (partition dim)
NCHUNKS = COLS // CCHUNK     # 32
SLAB_C = 4                   # uniform c-chunks per slab (bench variants)
# slab schedule: uniform 2.8MB DMAs for bandwidth, small final slabs so
# the compute tail after the last weight DMA is short
SLAB_SCHED = [4, 4, 4, 4, 4, 4, 4, 2, 2]
RBLOCKS = [(0, 512), (512, 512), (1024, 352)]

BF16 = ml_dtypes.bfloat16

_cached = {}


def _build_program(reps=1, loop_reps=0, mode="full", cast_dma=True,
                   slab_c=None, wbufs=3, dual=False, sched=None,
                   alt=False):
    """Build the device program. reps>1 repeats the streaming body
    (unrolled); loop_reps>0 wraps the body in a device-side For_i loop.
    mode: "full" | "dma_only" | "pe_only" (benchmark variants).
    cast_dma: SWDGE int32->bf16 cast in DMA; else HWDGE + DVE convert.
    slab_c: uniform slab size override; None uses SLAB_SCHED.
    dual: split each slab DMA across gpsimd/sync/scalar queues."""
    import concourse.mybir as mybir
    import concourse.tile as tile
    from concourse import bacc

    if sched is not None:
        sched = list(sched)
    elif slab_c is None:
        sched = list(SLAB_SCHED)
    else:
        sched = [slab_c] * (NCHUNKS // slab_c)
    assert sum(sched) == NCHUNKS
    max_slab = max(sched)

    nc = bacc.Bacc("TRN2", target_bir_lowering=False, debug=False,
                   enable_asserts=False, num_devices=NCORES)

    # weight shard, host-swizzled to the SBUF slab layout:
    # wT[p, k*RPC + r] = w^T[k*128 + p, r]  ->  every DMA is an identity
    # copy whose per-partition DRAM runs are slab_c*5504B contiguous
    wT = nc.dram_tensor("wT", [CCHUNK, NCHUNKS * RPC], mybir.dt.int32,
                        kind="ExternalInput").ap()
    xhi = nc.dram_tensor("xhi", [CCHUNK, NCHUNKS * T], mybir.dt.bfloat16,
                         kind="ExternalInput").ap()
    xlo = nc.dram_tensor("xlo", [CCHUNK, NCHUNKS * T], mybir.dt.bfloat16,
                         kind="ExternalInput").ap()
    bhi = nc.dram_tensor("bhi", [1, RPC], mybir.dt.bfloat16,
                         kind="ExternalInput").ap()
    blo = nc.dram_tensor("blo", [1, RPC], mybir.dt.bfloat16,
                         kind="ExternalInput").ap()
    out = nc.dram_tensor("out", [T, RPC], mybir.dt.float32,
                         kind="ExternalOutput").ap()

    with tile.TileContext(nc) as tc, ExitStack() as ctx:
        const = ctx.enter_context(tc.tile_pool(name="const", bufs=1))
        wpool = ctx.enter_context(tc.tile_pool(name="w", bufs=wbufs))
        psum = ctx.enter_context(tc.tile_pool(name="psum", bufs=3,
                                              space="PSUM"))
        opool = ctx.enter_context(tc.tile_pool(name="o", bufs=2))

        # x stationaries: [c_part, chunk*token], swizzled on host
        xhi_sb = const.tile([CCHUNK, COLS], mybir.dt.bfloat16, tag="xhi")
        xlo_sb = const.tile([CCHUNK, COLS], mybir.dt.bfloat16, tag="xlo")
        nc.sync.dma_start(out=xhi_sb[:], in_=xhi[:])
        nc.sync.dma_start(out=xlo_sb[:], in_=xlo[:])

        bhi_sb = const.tile([1, RPC], mybir.dt.bfloat16, tag="bhi")
        blo_sb = const.tile([1, RPC], mybir.dt.bfloat16, tag="blo")
        nc.sync.dma_start(out=bhi_sb[:], in_=bhi[:])
        nc.sync.dma_start(out=blo_sb[:], in_=blo[:])

        ones = const.tile([1, T], mybir.dt.bfloat16, tag="ones")
        nc.vector.memset(ones[:], 1.0)

        def body():
            ps = []
            if mode != "dma_only":
                # Seed each PSUM r-block with the bias (broadcast over
                # tokens by a K=1 matmul: ones^T [T] x bias [rn]).
                for r0, rn in RBLOCKS:
                    pt = psum.tile([T, rn], mybir.dt.float32, tag="acc")
                    ps.append(pt)
                    nc.tensor.matmul(pt[:], lhsT=ones[:],
                                     rhs=bhi_sb[:, r0:r0 + rn],
                                     start=True, stop=False)
                    nc.tensor.matmul(pt[:], lhsT=ones[:],
                                     rhs=blo_sb[:, r0:r0 + rn],
                                     start=False, stop=False)

            c0 = 0
            first_slab = None
            for s, sc in enumerate(sched):
                if mode == "pe_only" and s > 0:
                    wslab = first_slab
                    cur = sc
                else:
                    if dual:
                        # split the slab stream across the three DMA issue
                        # paths: SWDGE(cast) + 2x HWDGE(raw int32)
                        wslab = wpool.tile([CCHUNK, max_slab, RPC],
                                           mybir.dt.bfloat16, tag="wslab")
                        wraw = wpool.tile([CCHUNK, max_slab, RPC],
                                          mybir.dt.int32, tag="wraw")
                        src = wT[:, c0 * RPC:(c0 + sc) * RPC].rearrange(
                            "p (j r) -> p j r", r=RPC)
                        h = RPC // 2
                        q = h + RPC // 4
                        nc.gpsimd.dma_start(out=wslab[:, :sc, :h],
                                            in_=src[:, :, :h])
                        nc.sync.dma_start(out=wraw[:, :sc, h:q],
                                          in_=src[:, :, h:q])
                        nc.scalar.dma_start(out=wraw[:, :sc, q:],
                                            in_=src[:, :, q:])
                    elif alt and s % 2 == 1:
                        # odd slabs ride the HWDGE queue (raw int32) with a
                        # DVE convert, overlapping SWDGE issue overheads
                        wraw = wpool.tile([CCHUNK, max_slab, RPC],
                                          mybir.dt.int32, tag="wraw",
                                          bufs=2)
                        nc.sync.dma_start(
                            out=wraw[:, :sc, :],
                            in_=wT[:, c0 * RPC:(c0 + sc) * RPC])
                        wslab = wpool.tile([CCHUNK, max_slab, RPC],
                                           mybir.dt.bfloat16, tag="wslab")
                        nc.vector.tensor_copy(out=wslab[:, :sc, :],
                                              in_=wraw[:, :sc, :])
                    elif cast_dma:
                        if sc <= 2:
                            # tail slabs get dedicated slots so their DMAs
                            # never wait on big-slab buffer release (keeps
                            # the DMA queue streaming through the tail)
                            wslab = wpool.tile([CCHUNK, sc, RPC],
                                               mybir.dt.bfloat16,
                                               tag="wtail", bufs=2)
                        else:
                            wslab = wpool.tile([CCHUNK, max_slab, RPC],
                                               mybir.dt.bfloat16,
                                               tag="wslab")
                        # SWDGE cast-DMA: int32 DRAM -> bf16 SBUF
                        nc.gpsimd.dma_start(
                            out=wslab[:, :sc, :],
                            in_=wT[:, c0 * RPC:(c0 + sc) * RPC])
                    else:
                        wraw = wpool.tile([CCHUNK, max_slab, RPC],
                                          mybir.dt.int32, tag="wraw")
                        nc.sync.dma_start(
                            out=wraw[:, :sc, :],
                            in_=wT[:, c0 * RPC:(c0 + sc) * RPC])
                        wslab = wpool.tile([CCHUNK, max_slab, RPC],
                                           mybir.dt.bfloat16, tag="wslab")
                        nc.vector.tensor_copy(out=wslab[:, :sc, :],
                                              in_=wraw[:, :sc, :])
                    if mode == "pe_only" and s == 0:
                        first_slab = wslab
                if mode == "dma_only":
                    c0 += sc
                    continue
                for j in range(sc):
                    k = c0 + j
                    last_k = k == NCHUNKS - 1
                    if not last_k:
                        for x_sb, is_lo in ((xhi_sb, False), (xlo_sb, True)):
                            lhsT = x_sb[:, k * T:(k + 1) * T]
                            for rb, (r0, rn) in enumerate(RBLOCKS):
                                nc.tensor.matmul(
                                    ps[rb][:], lhsT=lhsT,
                                    rhs=wslab[:, j, r0:r0 + rn],
                                    start=False, stop=False)
                    else:
                        # final chunk: r-block-major so each PSUM bank hits
                        # its stop (and can drain) as early as possible
                        for rb, (r0, rn) in enumerate(RBLOCKS):
                            for x_sb, is_lo in ((xhi_sb, False),
                                                (xlo_sb, True)):
                                lhsT = x_sb[:, k * T:(k + 1) * T]
                                nc.tensor.matmul(
                                    ps[rb][:], lhsT=lhsT,
                                    rhs=wslab[:, j, r0:r0 + rn],
                                    start=False, stop=is_lo)
                c0 += sc

            if mode == "dma_only":
                return
            o_sb = opool.tile([T, RPC], mybir.dt.float32, tag="osb")
            drain = [nc.scalar.copy, nc.vector.tensor_copy, nc.scalar.copy]
            for rb, (r0, rn) in enumerate(RBLOCKS):
                drain[rb](out=o_sb[:, r0:r0 + rn], in_=ps[rb][:])
                nc.sync.dma_start(out=out[:, r0:r0 + rn],
                                  in_=o_sb[:, r0:r0 + rn])

        if loop_reps:
            with tc.For_i(0, loop_reps, 1):
                body()
        else:
            for _rep in range(reps):
                body()

    nc.compile()
    return nc


def _get_program():
    if "nc" not in _cached:
        _cached["nc"] = _build_program()
    return _cached["nc"]


def _prep_inputs(x, w_int8, bias):
    xs = (x.reshape(T, COLS).astype(np.float32) * np.float32(SCALE))
    xhi = xs.astype(BF16)
    xlo = (xs - xhi.astype(np.float32)).astype(BF16)

    def swizzle(a):
        # [T, COLS] -> [p, k*T + t] = x^T[k*128+p, t]: the exact SBUF
        # layout the PE stationary slices consume.
        return np.ascontiguousarray(
            a.reshape(T, NCHUNKS, CCHUNK).transpose(2, 1, 0)
        ).reshape(CCHUNK, NCHUNKS * T)

    xhi_dev = swizzle(xhi)
    xlo_dev = swizzle(xlo)

    bh = bias.astype(BF16)
    bl = (bias.astype(np.float32) - bh.astype(np.float32)).astype(BF16)
    bh_sh = np.ascontiguousarray(bh.reshape(NCORES, 1, RPC))
    bl_sh = np.ascontiguousarray(bl.reshape(NCORES, 1, RPC))

    # wT shards in SBUF slab layout: [core, p, k*RPC + r] = w[s*RPC + r,
    # k*128 + p] so device DMAs are identity copies with 22KB-contiguous
    # per-partition runs.
    w4 = w_int8.reshape(NCORES, RPC, NCHUNKS, CCHUNK)
    wT_sh = np.ascontiguousarray(w4.transpose(0, 3, 2, 1)).reshape(
        NCORES, CCHUNK, NCHUNKS * RPC)
    return xhi_dev, xlo_dev, bh_sh, bl_sh, wT_sh


def kernel(x, w_int8, bias):
    from concourse import bass_utils

    nc = _get_program()
    xhi_dev, xlo_dev, bh_sh, bl_sh, wT_sh = _prep_inputs(
        np.asarray(x), np.asarray(w_int8), np.asarray(bias))

    in_maps = [
        {"wT": wT_sh[s], "xhi": xhi_dev, "xlo": xlo_dev,
         "bhi": bh_sh[s], "blo": bl_sh[s]}
        for s in range(NCORES)
    ]
    res = bass_utils.run_bass_kernel_spmd(nc, in_maps,
                                          core_ids=list(range(NCORES)))
    shards = [res.results[s]["out"] for s in range(NCORES)]
    y = np.concatenate(shards, axis=1).reshape(4, 32, ROWS)
    return np.ascontiguousarray(y.astype(np.float32))



# revision 2
# speedup vs baseline: 1.9716x; 1.9716x over previous
"""CompressedLinear Trainium2 kernel.

Computes y = x @ (w_int8 * 0.01)^T + bias for
  x      [4, 32, 4096]  fp32
  w_int8 [11008, 4096]  int32 (int8 values)
  bias   [11008]        fp32
  y      [4, 32, 11008] fp32

Strategy (tensor-parallel over output rows, 8 NeuronCores):
- The weight payload is int8; stream it from HBM as 1 byte/element
  (5.6 MB/core instead of the baseline's 22.5 MB int32) and widen to
  fp16 on-chip. A single fp16 matmul pass suffices: int8 weights are
  exact in fp16 and x's fp16 rounding gives rel err ~2e-4 (tolerance
  is 2e-2). This drops the kernel from HBM-bound (63 us floor) to
  PE-bound (~18.4 us floor = 32 chunks x 1376 rows / 2.4 GHz).
- The int8->fp16 widening is split across three engines so no single
  path binds: rows [0,F) via SWDGE cast-DMA (the DMA datapath converts
  inline; SBUF write side costs 2 B/elem of the 435 GB/s fabric),
  rows [F,F+D) via DVE tensor_copy (0.96 GHz, 1 col/cycle), rows
  [F+D,RPC) via ACT copy (1.2 GHz, 1 col/cycle). The raw int8 slab for
  the engine-converted rows rides HWDGE at 1 B/elem.
- Host: fold the 0.01 scale into x (fp16), transpose/swizzle x and w
  into the exact SBUF layouts the device consumes, split w rows into
  the cast/raw groups so every DMA is a fully contiguous identity copy.
- Device, per core: stream weight slabs (a few c-chunks each); per
  128-wide contraction chunk, load x^T as PE stationary and stream the
  fp16 weight rows in bank-aligned segments; accumulate y[tokens,rows]
  in 3 PSUM banks (512/512/352). Bias seeds PSUM via K=1 matmuls
  against a ones-vector (fp16, error ~4e-5 of max|y|) before the first
  weight slab lands, so it costs nothing. Per-bank drains alternate
  ScalarE/VectorE; each bank's output DMA starts when its drain lands.
- Host: concatenate the 8 row-shards, reshape.
"""

from contextlib import ExitStack

import numpy as np

ROWS, COLS = 11008, 4096
SCALE = 0.01
T = 128                      # tokens = 4*32
NCORES = 8
RPC = ROWS // NCORES         # 1376 rows per core
CCHUNK = 128                 # contraction tile (partition dim)
NCHUNKS = COLS // CCHUNK     # 32
F_CAST = 352                 # rows widened by SWDGE cast-DMA
D_DVE = 464                  # rows widened by DVE tensor_copy
# remaining RPC - F_CAST - D_DVE = 560 rows widened by ACT copy
SLAB_SCHED = [2, 2, 4, 4, 4, 4, 4, 4, 4]
RBLOCKS = [(0, 512), (512, 512), (1024, 352)]

_cached = {}


def _pieces(f, d):
    """Bank-aligned PE stream segments: (src, lo, hi, bank, boff) where
    src in {c,d,a}, [lo,hi) is the tile-local column range."""
    segs = [("c", 0, f), ("d", f, f + d), ("a", f + d, RPC)]
    base = {"c": 0, "d": f, "a": f + d}
    out = []
    for kind, g0, g1 in segs:
        for b, (r0, rn) in enumerate(RBLOCKS):
            lo, hi = max(g0, r0), min(g1, r0 + rn)
            if lo < hi:
                out.append((kind, lo - base[kind], hi - base[kind], b, lo - r0))
    last_for_bank = {}
    for i, p in enumerate(out):
        last_for_bank[p[3]] = i
    return out, last_for_bank


def _build_program(reps=1, loop_reps=0, mode="full", f_cast=F_CAST,
                   d_dve=D_DVE, sched=None, wbufs=3,
                   drain_eng=("scalar", "vector", "scalar")):
    """Build the device program. reps>1 repeats the streaming body
    (unrolled); loop_reps>0 wraps the body in a device-side For_i loop.
    mode: "full" | "dma_only" (stream weights, no compute)."""
    import concourse.mybir as mybir
    import concourse.tile as tile
    from concourse import bacc

    sched = list(SLAB_SCHED if sched is None else sched)
    assert sum(sched) == NCHUNKS
    max_slab = max(sched)
    f, dd = f_cast, d_dve
    aa = RPC - f - dd
    pieces, last_for_bank = _pieces(f, dd)

    nc = bacc.Bacc("TRN2", target_bir_lowering=False, debug=False,
                   enable_asserts=False, num_devices=NCORES)

    # weight shards, host-swizzled to the SBUF slab layout:
    # w*[p, k*R + r] = w^T[k*128 + p, row_group_base + r] -> every DMA is
    # an identity copy with fully contiguous per-partition DRAM runs.
    wc = nc.dram_tensor("wc", [CCHUNK, NCHUNKS * f], mybir.dt.int8,
                        kind="ExternalInput").ap()
    wr = nc.dram_tensor("wr", [CCHUNK, NCHUNKS * (dd + aa)], mybir.dt.int8,
                        kind="ExternalInput").ap()
    x16 = nc.dram_tensor("x16", [CCHUNK, NCHUNKS * T], mybir.dt.float16,
                         kind="ExternalInput").ap()
    b16 = nc.dram_tensor("b16", [1, RPC], mybir.dt.float16,
                         kind="ExternalInput").ap()
    out = nc.dram_tensor("out", [T, RPC], mybir.dt.float32,
                         kind="ExternalOutput").ap()

    RR = dd + aa

    with tile.TileContext(nc) as tc, ExitStack() as ctx:
        const = ctx.enter_context(tc.tile_pool(name="const", bufs=1))
        wcp = ctx.enter_context(tc.tile_pool(name="wc", bufs=wbufs))
        wrp = ctx.enter_context(tc.tile_pool(name="wr", bufs=wbufs))
        wdp = ctx.enter_context(tc.tile_pool(name="wd", bufs=wbufs))
        wap = ctx.enter_context(tc.tile_pool(name="wa", bufs=wbufs))
        psum = ctx.enter_context(tc.tile_pool(name="psum", bufs=3,
                                              space="PSUM"))
        opool = ctx.enter_context(tc.tile_pool(name="o", bufs=2))

        # x stationary: [c_part, chunk*token], swizzled on host
        x_sb = const.tile([CCHUNK, COLS], mybir.dt.float16, tag="x")
        nc.sync.dma_start(out=x_sb[:], in_=x16[:])
        b_sb = const.tile([1, RPC], mybir.dt.float16, tag="b")
        nc.sync.dma_start(out=b_sb[:], in_=b16[:])
        ones = const.tile([1, T], mybir.dt.float16, tag="ones")
        nc.vector.memset(ones[:], 1.0)

        drains = {"scalar": nc.scalar.copy, "vector": nc.vector.tensor_copy}

        def body():
            ps = []
            o_sb = None
            if mode == "full":
                # Seed each PSUM r-block with the bias (broadcast over
                # tokens by a K=1 matmul: ones^T [T] x bias [rn]). Runs
                # while the first weight slab is still in flight.
                for r0, rn in RBLOCKS:
                    pt = psum.tile([T, rn], mybir.dt.float32, tag="acc")
                    ps.append(pt)
                    nc.tensor.matmul(pt[:], lhsT=ones[:],
                                     rhs=b_sb[:, r0:r0 + rn],
                                     start=True, stop=False)
                o_sb = opool.tile([T, RPC], mybir.dt.float32, tag="osb")

            c0 = 0
            for s, sc in enumerate(sched):
                wc_sb = wcp.tile([CCHUNK, max_slab, f], mybir.dt.float16,
                                 tag="wc")
                # SWDGE cast-DMA: int8 DRAM -> fp16 SBUF
                nc.gpsimd.dma_start(out=wc_sb[:, :sc, :],
                                    in_=wc[:, c0 * f:(c0 + sc) * f])
                wr_sb = wrp.tile([CCHUNK, max_slab, RR], mybir.dt.int8,
                                 tag="wr")
                nc.sync.dma_start(out=wr_sb[:, :sc, :],
                                  in_=wr[:, c0 * RR:(c0 + sc) * RR])
                wd_sb = wdp.tile([CCHUNK, max_slab, dd], mybir.dt.float16,
                                 tag="wd")
                nc.vector.tensor_copy(out=wd_sb[:, :sc, :],
                                      in_=wr_sb[:, :sc, :dd])
                wa_sb = wap.tile([CCHUNK, max_slab, aa], mybir.dt.float16,
                                 tag="wa")
                nc.scalar.copy(out=wa_sb[:, :sc, :],
                               in_=wr_sb[:, :sc, dd:])
                if mode != "dma_only":
                    src = {"c": wc_sb, "d": wd_sb, "a": wa_sb}
                    for j in range(sc):
                        k = c0 + j
                        lhsT = x_sb[:, k * T:(k + 1) * T]
                        for i, (kind, lo, hi, b, boff) in enumerate(pieces):
                            stop = (k == NCHUNKS - 1
                                    and last_for_bank[b] == i)
                            nc.tensor.matmul(
                                ps[b][:, boff:boff + hi - lo], lhsT=lhsT,
                                rhs=src[kind][:, j, lo:hi],
                                start=False, stop=stop)
                            if stop:
                                # drain the bank the moment it stops;
                                # its output DMA follows immediately
                                r0, rn = RBLOCKS[b]
                                drains[drain_eng[b]](
                                    out=o_sb[:, r0:r0 + rn], in_=ps[b][:])
                                nc.sync.dma_start(out=out[:, r0:r0 + rn],
                                                  in_=o_sb[:, r0:r0 + rn])
                c0 += sc

        if loop_reps:
            with tc.For_i(0, loop_reps, 1):
                body()
        else:
            for _rep in range(reps):
                body()

    nc.compile()
    return nc


def _get_program():
    if "nc" not in _cached:
        _cached["nc"] = _build_program()
    return _cached["nc"]


def _prep_inputs(x, w_int8, bias, f_cast=F_CAST):
    xs = (x.reshape(T, COLS).astype(np.float32) * np.float32(SCALE))
    # [T, COLS] -> [p, k*T + t] = x^T[k*128+p, t]: the exact SBUF layout
    # the PE stationary slices consume.
    x_dev = np.ascontiguousarray(
        xs.astype(np.float16).reshape(T, NCHUNKS, CCHUNK).transpose(2, 1, 0)
    ).reshape(CCHUNK, NCHUNKS * T)

    b_sh = np.ascontiguousarray(
        bias.astype(np.float16).reshape(NCORES, 1, RPC))

    # per-core row groups in SBUF slab layout: [core, p, k*R + r] =
    # w[s*RPC + g0 + r, k*128 + p]
    w8 = w_int8.astype(np.int8).reshape(NCORES, RPC, NCHUNKS, CCHUNK)
    f = f_cast
    wc_sh = np.ascontiguousarray(w8[:, :f].transpose(0, 3, 2, 1)).reshape(
        NCORES, CCHUNK, NCHUNKS * f)
    wr_sh = np.ascontiguousarray(w8[:, f:].transpose(0, 3, 2, 1)).reshape(
        NCORES, CCHUNK, NCHUNKS * (RPC - f))
    return x_dev, b_sh, wc_sh, wr_sh


def kernel(x, w_int8, bias):
    from concourse import bass_utils

    nc = _get_program()
    x_dev, b_sh, wc_sh, wr_sh = _prep_inputs(
        np.asarray(x), np.asarray(w_int8), np.asarray(bias))

    in_maps = [
        {"wc": wc_sh[s], "wr": wr_sh[s], "x16": x_dev, "b16": b_sh[s]}
        for s in range(NCORES)
    ]
    res = bass_utils.run_bass_kernel_spmd(nc, in_maps,
                                          core_ids=list(range(NCORES)))
    shards = [res.results[s]["out"] for s in range(NCORES)]
    y = np.concatenate(shards, axis=1).reshape(4, 32, ROWS)
    return np.ascontiguousarray(y.astype(np.float32))
